# revision 2
# baseline (speedup 1.0000x reference)
"""Dense dot-product attention (B=8, S=2048, D=64, fp32) on 8 TRN2 NeuronCores.

Sharding: batch dim across the 8 cores (data parallel), one batch element per
core. v2 design (bf16 matmul path, fp32 softmax/epilogue):

  Host prep: Q/K are zero-padded to [S, 128] and cast to bf16 (XBAR DMA
  transpose needs free%128==0 and a 2-byte dtype); V cast to bf16.

  Q^T/K^T land in SBUF directly via hardware XBAR transpose DMAs (no PE
  transposes, no casts). The key padding mask enters through the exp bias
  (per-partition AP on the ACT engine), so the score matmul is a clean
  64-contraction with 128-wide bf16 weights -> fast weight load.

  Main loop per k-chunk n (128 rows), per q-halftile e (1024 cols):
    ST[k,q] = K_chunk @ Q^T (2 x 512-col matmuls, bf16, PSUM fp32)
    se = exp(0.125*ST + maskbias)  on ACT, bf16 out
    PV[d,q] += V'_chunk^T @ se     (V' = [v*mask_v | 1 | 0-pad], 128 wide)
  ACT does nothing but the 32 exps: it is the 35.5us roofline of the loop.

  Epilogue: pv[0:65,:] -> SBUF (ACT/DVE halves), 16 PE transposes into one
  padded PSUM tile, ONE strided DVE reciprocal for all 16 denominators,
  per-m multiplies split DVE/ACT, output DMA in two halves on two queues.
"""

import numpy as np
import ml_dtypes

import concourse.bass as bass
import concourse.mybir as mybir
import concourse.tile as tile
from concourse import bacc
from concourse.bass import ts
from concourse.bass_utils import run_bass_kernel_spmd
from concourse.masks import make_identity

B, S, D = 8, 2048, 64
NEG = -1e9
P = 128
NKC = S // P     # 16 k-chunks
EW = 1024        # exp granularity (q width per ST tile)
NE = S // EW     # ST tiles per chunk
MMW = 512        # matmul moving width (one fp32 PSUM bank)
F32 = mybir.dt.float32
BF16 = mybir.dt.bfloat16
BF = ml_dtypes.bfloat16

_CACHE: dict = {}


def _build_nc():
    # Bacc (not raw Bass): its compile() splits multi-wait sync lists into
    # event semaphores - TRN2 instructions carry at most ONE sync wait.
    nc = bacc.Bacc("TRN2", target_bir_lowering=False, debug=False)

    qp = nc.dram_tensor("qp", [S, P], BF16, kind="ExternalInput").ap()
    kp = nc.dram_tensor("kp", [S, P], BF16, kind="ExternalInput").ap()
    vn = nc.dram_tensor("vn", [S, D], BF16, kind="ExternalInput").ap()
    mk = nc.dram_tensor("mk", [S], F32, kind="ExternalInput").ap()
    mv = nc.dram_tensor("mv", [S], F32, kind="ExternalInput").ap()
    out = nc.dram_tensor("out", [S, D], F32, kind="ExternalOutput").ap()

    with tile.TileContext(nc) as tc:
        with (
            tc.tile_pool(name="const", bufs=1) as const,
            tc.tile_pool(name="se", bufs=3) as se_pool,
        ):
            ident = const.tile([P, P], F32)
            make_identity(nc, ident)

            qt = const.tile([P, S], BF16, tag="qt")
            kt = const.tile([P, S], BF16, tag="kt")
            vf = const.tile([P, NKC, D], BF16, tag="vf")
            vp = const.tile([P, NKC, P], BF16, tag="vp")
            mks = const.tile([P, NKC], F32, tag="mks")
            mkb = const.tile([P, NKC], F32, tag="mkb")
            mvs = const.tile([P, NKC], F32, tag="mvs")
            ob = const.tile([P, NKC, D], F32, tag="ob")
            pvsb = const.tile([D + 1, S], F32, tag="pvsb")
            recs = const.tile([P, NKC], F32, tag="recs")

            # Input DMAs. k/masks on the sync queue, q/v on the scalar
            # queue so descriptor generation overlaps. Halved transposes so
            # chunk 0 of both Q^T and K^T lands ~1us earlier.
            HS = S // 2
            nc.sync.dma_start_transpose(out=kt[:, 0:HS], in_=kp[0:HS, :])
            nc.scalar.dma_start_transpose(out=qt[:, 0:HS], in_=qp[0:HS, :])
            nc.sync.dma_start(out=mks, in_=mk.rearrange("(n p) -> p n", p=P))
            nc.sync.dma_start_transpose(out=kt[:, HS:S], in_=kp[HS:S, :])
            nc.scalar.dma_start_transpose(out=qt[:, HS:S], in_=qp[HS:S, :])
            nc.sync.dma_start(out=mvs, in_=mv.rearrange("(n p) -> p n", p=P))
            nc.scalar.dma_start(out=vf, in_=vn.rearrange("(n p) d -> p n d", p=P))

            # Key-mask additive bias rides in the exp: bias = (mk-1)*1e9.
            nc.gpsimd.tensor_scalar(
                mkb, mks, -NEG, NEG,
                op0=mybir.AluOpType.mult, op1=mybir.AluOpType.add,
            )

            # V' chunks: [128, 128] with cols 0:64 = V*mask_v, col 64 = 1.0,
            # cols 65:128 = 0 (padding so NumWeights==128 enables FWL; the
            # extra pv output partitions are never read).
            nc.gpsimd.memset(vp[:, :, D + 1 : P], 0.0)
            nc.gpsimd.memset(vp[:, :, D : D + 1], 1.0)
            for n in range(NKC):
                nc.vector.tensor_scalar(
                    vp[:, n, 0:D], vf[:, n, :], mvs[:, n : n + 1], None,
                    op0=mybir.AluOpType.mult,
                )

            # Main loop: ST tile -> exp -> PV accumulate
            with tc.tile_pool(name="pv_ps", bufs=1, space="PSUM") as pv_ps:
                pv = pv_ps.tile([P, S], F32, tag="pv")
                with tc.tile_pool(name="st_ps", bufs=2, space="PSUM") as st_ps:
                    for n in range(NKC):
                        for e in range(NE):
                            st = st_ps.tile([P, EW], F32, tag="st")
                            for h in range(EW // MMW):
                                nc.tensor.matmul(
                                    st[:, ts(h, MMW)],
                                    lhsT=kt[0:D, ts(n, P)],
                                    rhs=qt[0:D, ts(e * (EW // MMW) + h, MMW)],
                                    start=True,
                                    stop=True,
                                )
                            se = se_pool.tile([P, EW], BF16, tag="se")
                            nc.scalar.activation(
                                se, st, mybir.ActivationFunctionType.Exp,
                                bias=mkb[:, n : n + 1], scale=0.125,
                            )
                            for h in range(EW // MMW):
                                nc.tensor.matmul(
                                    pv[:, ts(e * (EW // MMW) + h, MMW)],
                                    lhsT=vp[:, n, :],
                                    rhs=se[:, ts(h, MMW)],
                                    start=(n == 0),
                                    stop=(n == NKC - 1),
                                )

                # Epilogue: pv -> SBUF, transpose to [q, d+denom] into one
                # padded PSUM tile, one strided reciprocal, scaled copies out.
                with tc.tile_pool(name="ep_ps", bufs=1, space="PSUM") as ep_ps:
                    otall = ep_ps.tile([P, NKC, P], F32, tag="ot")
                    CG = 4
                    W = S // CG
                    for g in range(CG):
                        sl = slice(g * W, (g + 1) * W)
                        if g % 2 == 0:
                            nc.scalar.copy(pvsb[:, sl], pv[0 : D + 1, sl])
                        else:
                            nc.vector.tensor_copy(pvsb[:, sl], pv[0 : D + 1, sl])
                    for m in range(NKC):
                        nc.tensor.transpose(
                            otall[:, m, 0 : D + 1],
                            pvsb[:, ts(m, P)],
                            ident[0 : D + 1, 0 : D + 1],
                        )
                    nc.vector.reciprocal(recs, otall[:, :, D])
                    for m in range(NKC):
                        if m % 2 == 0:
                            nc.vector.tensor_scalar(
                                ob[:, m, :], otall[:, m, 0:D],
                                recs[:, m : m + 1], None,
                                op0=mybir.AluOpType.mult,
                            )
                        else:
                            nc.scalar.mul(
                                ob[:, m, :], otall[:, m, 0:D],
                                recs[:, m : m + 1],
                            )
                    orr = out.rearrange("(n p) d -> p n d", p=P)
                    HC = NKC // 2
                    nc.scalar.dma_start(out=orr[:, 0:HC, :], in_=ob[:, 0:HC, :])
                    nc.sync.dma_start(out=orr[:, HC:NKC, :], in_=ob[:, HC:NKC, :])

    nc.compile()
    return nc


def get_nc():
    if "nc" not in _CACHE:
        _CACHE["nc"] = _build_nc()
    return _CACHE["nc"]


def _in_maps(queries, keys, values, mask_k, mask_v):
    qpad = np.zeros((B, S, P), dtype=np.float32)
    qpad[:, :, 0:D] = queries
    kpad = np.zeros((B, S, P), dtype=np.float32)
    kpad[:, :, 0:D] = keys
    qpb = qpad.astype(BF)
    kpb = kpad.astype(BF)
    vb = np.asarray(values, dtype=np.float32).astype(BF)
    mkf = np.asarray(mask_k, dtype=np.float32)
    mvf = np.asarray(mask_v, dtype=np.float32)
    return [
        {
            "qp": np.ascontiguousarray(qpb[b]),
            "kp": np.ascontiguousarray(kpb[b]),
            "vn": np.ascontiguousarray(vb[b]),
            "mk": np.ascontiguousarray(mkf[b]),
            "mv": np.ascontiguousarray(mvf[b]),
        }
        for b in range(B)
    ]


def kernel(queries, keys, values, mask_q, mask_k, mask_v, **_unused):
    nc = get_nc()
    in_maps = _in_maps(queries, keys, values, mask_k, mask_v)
    res = run_bass_kernel_spmd(nc, in_maps, core_ids=list(range(B)))
    return np.stack([res.results[b]["out"] for b in range(B)], axis=0)
